# revision 15
# baseline (speedup 1.0000x reference)
"""Trainium2 Bass kernel for EnhancedPrototypeMemory (scatter_memory).

Strategy (8 NeuronCores, data-parallel over N):
  - Each core streams its N/8 = 16384 feature rows once from HBM with an
    on-the-fly f32->fp16 cast in the SWDGE DMA (full 128 MiB still read from
    HBM; SBUF tiles and PE operands are fp16).
  - Per 128-row tile: row sum-of-squares split across engines (a fraction of
    tiles uses ACT Square+accumulate, the rest uses a batched ACT Square into
    scratch + one batched DVE reduce) -> sqrt -> reciprocal; a one-hot matrix
    scaled by 1/||f|| is built on DVE (fused is_equal+mult tensor_scalar
    against an iota constant).
  - PE matmul (fp16 -> fp32 PSUM) accumulates onehot_scaled.T @
    [features | norm] into a [91,257] PSUM tile: cols 0..255 = normalized
    per-class sums, col 256 = per-class count (rnorm*norm = 1 per member);
    counts are re-rounded to exact integers in the epilogue.
  - The [91,257] partials are AllReduce'd across the 8 cores, then every
    core applies the identical EMA epilogue on [91,256] tiles.
  - All six epilogue state inputs are packed into ONE [91,516] input DMA and
    all six outputs into ONE [91,516] output DMA (single completion receipt).
Host only shards/packs/unpacks and converts bool/int masks to f32 and back.
"""

import numpy as np

import concourse.bacc as bacc
import concourse.mybir as mybir
import concourse.tile as tile
from concourse.bass_utils import run_bass_kernel_spmd

# Problem constants (hardcoded per contract; kernel.py must be self-contained).
N_CORES = 8
C = 91
D = 256
N_TOTAL = 131072
N_SHARD = N_TOTAL // N_CORES  # 16384
P = 128
T = N_SHARD // P  # 128 tiles per core
MOMENTUM = 0.999
WARMUP_STEPS = 200
BASE_MOM = 0.99
SHADOW_M = min(MOMENTUM + 0.0009, 0.9999)  # 0.9999

# Tunables.
GROUPS = [2, 2, 4, 8] + [16] * 7  # feature tiles per DMA group (sums to T=128)
G_MAX = max(GROUPS)
F_BUFS = 6       # feature-tile buffering

def _r_red(g):
    # tiles per group on the batched-square + DVE-reduce path; the rest use
    # ACT Square+accumulate  (~0.56 of the group keeps ACT and DVE balanced)
    return max(1, (g * 9 + 8) // 16)

# Packed epilogue state layout (both input and output): [91, 516]
#   0:256   prototypes            -> new_proto
#   256:512 shadow_prototypes     -> new_shadow
#   512     proto_variance        -> new_var
#   513     proto_initialized     -> new_init
#   514     shadow_initialized    -> new_sinit
#   515     proto_update_count    -> new_count
W = 2 * D + 4  # 516
CPAD = 96      # classes padded to 8*12 for the ReduceScatter
CS = CPAD // N_CORES  # 12-class shard per core

FP = mybir.dt.float32
FH = mybir.dt.float16
AF = mybir.ActivationFunctionType
OP = mybir.AluOpType

_cache: dict = {}


def _build(progress: float):
    nc = bacc.Bacc("TRN2", target_bir_lowering=False, debug=False,
                   num_devices=N_CORES)

    feat = nc.dram_tensor("feat", [N_SHARD, D], FP, kind="ExternalInput")
    labels_t = nc.dram_tensor("labels_t", [P, T], FP, kind="ExternalInput")
    epi_in = nc.dram_tensor("epi_in", [CS, W], FP, kind="ExternalInput")
    out_all = nc.dram_tensor("out_all", [CS, W], FP, kind="ExternalOutput")

    iota_const = nc.inline_tensor(
        np.tile(np.arange(C, dtype=np.float32), (P, 1)).astype(np.float16),
        name="iota_const")

    k_mom = float(np.float32(MOMENTUM - BASE_MOM))

    with tile.TileContext(nc) as tc:
        with (
            tc.tile_pool(name="const", bufs=1) as cpool,
            tc.tile_pool(name="feat", bufs=F_BUFS) as fpool,
            tc.tile_pool(name="oh", bufs=12) as ohpool,
            tc.tile_pool(name="stats", bufs=8) as spool,
            tc.tile_pool(name="scratch", bufs=3) as scrpool,
            tc.tile_pool(name="psum", bufs=1, space="PSUM") as pspool,
            tc.tile_pool(name="epi", bufs=1) as epool,
            tc.tile_pool(name="dram", bufs=1, space="DRAM") as dram,
        ):
            feat_r = feat[:, :].rearrange("(t p) d -> p t d", p=P)

            # First group's feature load goes first so DMA starts immediately.
            G0 = GROUPS[0]
            ftile0 = fpool.tile([P, G0, D + 1], FH, tag="ftile", name="ftile0",
                                padded_shape=[P, G_MAX, D + 1])
            nc.gpsimd.dma_start(ftile0[:, :, 0:D], feat_r[:, 0:G0, :])

            # Tiny warm-up AllGather: pays the collective entry barrier and
            # absorbs inter-core launch skew while the main body runs.
            warm_sb = cpool.tile([8, 4], FP, tag="warm_sb")
            nc.vector.memset(warm_sb[:], 1.0)
            wu_in = dram.tile([8, 4], FP, tag="wu_in")
            wu_out = dram.tile([N_CORES * 8, 4], FP, tag="wu_out")
            nc.sync.dma_start(wu_in[:], warm_sb[:])
            nc.gpsimd.collective_compute(
                "AllGather", OP.bypass,
                replica_groups=[list(range(N_CORES))],
                ins=[wu_in[:].opt()], outs=[wu_out[:].opt()])

            # One-time constants / small inputs.
            iota_sb = cpool.tile([P, C], FH, tag="iota")
            nc.sync.dma_start(iota_sb[:], iota_const[:, :])
            labels_sb = cpool.tile([P, T], FP, tag="labels")
            nc.sync.dma_start(labels_sb[:], labels_t[:, :])
            epi_sb = epool.tile([CS, W], FP, tag="epi_sb")
            nc.sync.dma_start(epi_sb[:], epi_in[:, :])

            proto_sb = epi_sb[:, 0:D]
            shadow_sb = epi_sb[:, D:2 * D]
            var_sb = epi_sb[:, 2 * D:2 * D + 1]
            init_sb = epi_sb[:, 2 * D + 1:2 * D + 2]
            sinit_sb = epi_sb[:, 2 * D + 2:2 * D + 3]
            count_sb = epi_sb[:, 2 * D + 3:2 * D + 4]

            psum = pspool.tile([C, D + 1], FP, tag="acc")

            g0 = 0
            for gi, G in enumerate(GROUPS):
                R = _r_red(G)
                if gi == 0:
                    ftile = ftile0
                else:
                    ftile = fpool.tile([P, G, D + 1], FH, tag="ftile",
                                       name=f"ftile{gi}",
                                       padded_shape=[P, G_MAX, D + 1])
                    nc.gpsimd.dma_start(ftile[:, :, 0:D],
                                        feat_r[:, g0:g0 + G, :])

                sumsq = spool.tile([P, G], FP, tag="sumsq",
                                   padded_shape=[P, G_MAX])
                # tiles 0..R-1: one batched ACT square + one DVE reduce
                sq_scr = scrpool.tile([P, R, D], FH, tag="sq_scr",
                                      padded_shape=[P, _r_red(G_MAX), D])
                nc.scalar.activation(sq_scr[:], ftile[:, 0:R, 0:D], AF.Square)
                nc.vector.tensor_reduce(sumsq[:, 0:R], sq_scr[:],
                                        axis=mybir.AxisListType.X, op=OP.add)
                # tiles R..G-1: ACT square+accumulate
                sq_scr2 = scrpool.tile([P, D], FH, tag="sq_scr2")
                for g in range(R, G):
                    nc.scalar.activation(
                        sq_scr2[:], ftile[:, g, 0:D], AF.Square,
                        accum_out=sumsq[:, g:g + 1])

                norms = spool.tile([P, G], FP, tag="norms",
                                   padded_shape=[P, G_MAX])
                nc.scalar.activation(norms[:], sumsq[:], AF.Sqrt)
                nc.vector.tensor_copy(ftile[:, :, D:D + 1], norms[:])
                rnorm = spool.tile([P, G], FP, tag="rnorm",
                                   padded_shape=[P, G_MAX])
                nc.vector.reciprocal(rnorm[:], norms[:])

                for g in range(G):
                    t = g0 + g
                    oh = ohpool.tile([P, C], FH, tag="oh")
                    nc.vector.tensor_scalar(
                        oh[:], iota_sb[:],
                        labels_sb[:, t:t + 1], rnorm[:, g:g + 1],
                        op0=OP.is_equal, op1=OP.mult)
                    nc.tensor.matmul(
                        psum[:], oh[:], ftile[:, g, :],
                        start=(t == 0), stop=(t == T - 1))
                g0 += G

            # ---- cross-core ReduceScatter of [96, 258] padded partials ----
            # rank r receives the summed class slice [12r : 12r+12]; the
            # epilogue below runs on that 12-class shard and the host
            # concatenates the 8 output slices.
            partial = epool.tile([CPAD, D + 2], FP, tag="partial")
            nc.vector.memset(partial[:], 0.0)
            nc.scalar.copy(partial[0:C, 0:D + 1], psum[:])
            cc_in = dram.tile([CPAD, D + 2], FP, tag="cc_in")
            rs_out = dram.tile([CS, D + 2], FP, tag="rs_out")
            nc.sync.dma_start(cc_in[:], partial[:])
            nc.gpsimd.collective_compute(
                "ReduceScatter", OP.add,
                replica_groups=[list(range(N_CORES))],
                ins=[cc_in[:].opt()], outs=[rs_out[:].opt()])
            total = epool.tile([CS, D + 2], FP, tag="total")
            nc.sync.dma_start(total[:], rs_out[:, :])

            # ---- epilogue (on this core's 12-class shard) ----
            sums = total[:, 0:D]
            counts_raw = total[:, D:D + 1]

            out_sb = epool.tile([CS, W], FP, tag="out_sb")
            newp = out_sb[:, 0:D]
            news = out_sb[:, D:2 * D]
            newv = out_sb[:, 2 * D:2 * D + 1]
            newi = out_sb[:, 2 * D + 1:2 * D + 2]
            newsi = out_sb[:, 2 * D + 2:2 * D + 3]
            newc = out_sb[:, 2 * D + 3:2 * D + 4]

            def etile(tag, shape=(CS, 1)):
                return epool.tile(list(shape), FP, tag=tag, name=tag)

            # round fp16-accumulated counts to exact ints (2^23 trick)
            counts = etile("counts")
            nc.vector.tensor_scalar(counts[:], counts_raw, float(2 ** 23),
                                    float(-(2 ** 23)), op0=OP.add, op1=OP.add)

            present = etile("present")
            nc.vector.tensor_scalar(present[:], counts[:], 0.0, None,
                                    op0=OP.is_gt)
            cntc = etile("cntc")
            nc.vector.tensor_scalar(cntc[:], counts[:], 1.0, None, op0=OP.max)
            inv = etile("inv")
            nc.vector.reciprocal(inv[:], cntc[:])

            # alpha = present * (1 - init*mom); mom = BASE + k*progress*e^-var
            e = etile("e")
            nc.scalar.activation(e[:], var_sb, AF.Exp, scale=-1.0)
            momt = etile("momt")
            nc.vector.tensor_scalar(momt[:], e[:], float(k_mom * progress),
                                    init_sb, op0=OP.mult, op1=OP.mult)
            a1 = etile("a1")
            nc.vector.tensor_scalar(a1[:], init_sb, float(-BASE_MOM), 1.0,
                                    op0=OP.mult, op1=OP.add)
            a2 = etile("a2")
            nc.vector.tensor_tensor(a2[:], a1[:], momt[:], op=OP.subtract)
            alpha = etile("alpha")
            nc.vector.tensor_tensor(alpha[:], a2[:], present[:], op=OP.mult)

            d = etile("d", (CS, D))  # cls_feat - old = sums*inv - old
            nc.vector.scalar_tensor_tensor(d[:], sums, inv[:], proto_sb,
                                           op0=OP.mult, op1=OP.subtract)
            nc.vector.scalar_tensor_tensor(newp, d[:], alpha[:], proto_sb,
                                           op0=OP.mult, op1=OP.add)

            sq91 = etile("sq91", (CS, D))
            ss = etile("ss")
            nc.scalar.activation(sq91[:], d[:], AF.Square, accum_out=ss[:])
            mag = etile("mag")
            nc.scalar.activation(mag[:], ss[:], AF.Sqrt)

            w_m = etile("w_m")
            nc.vector.tensor_scalar(w_m[:], present[:], init_sb, 0.01,
                                    op0=OP.mult, op1=OP.mult)
            g2 = etile("g2")
            nc.vector.tensor_tensor(g2[:], mag[:], var_sb, op=OP.subtract)
            wg = etile("wg")
            nc.vector.tensor_tensor(wg[:], g2[:], w_m[:], op=OP.mult)
            nc.vector.tensor_tensor(newv, var_sb, wg[:], op=OP.add)

            b1 = etile("b1")
            nc.vector.tensor_scalar(b1[:], sinit_sb, float(-SHADOW_M), 1.0,
                                    op0=OP.mult, op1=OP.add)
            beta = etile("beta")
            nc.vector.tensor_tensor(beta[:], b1[:], present[:], op=OP.mult)
            d2 = etile("d2", (CS, D))
            nc.vector.tensor_tensor(d2[:], newp, shadow_sb, op=OP.subtract)
            nc.vector.scalar_tensor_tensor(news, d2[:], beta[:], shadow_sb,
                                           op0=OP.mult, op1=OP.add)

            nc.vector.tensor_tensor(newi, init_sb, present[:], op=OP.max)
            nc.vector.tensor_tensor(newsi, sinit_sb, present[:], op=OP.max)
            nc.vector.tensor_tensor(newc, count_sb, present[:], op=OP.add)

            nc.sync.dma_start(out_all[:, :], out_sb[:])

    nc.finalize()
    return nc


def kernel(features, labels, prototypes, proto_initialized, proto_variance,
           shadow_prototypes, shadow_initialized, proto_update_count, step):
    features = np.ascontiguousarray(np.asarray(features, dtype=np.float32))
    labels = np.asarray(labels)
    prototypes = np.asarray(prototypes, dtype=np.float32)
    proto_initialized = np.asarray(proto_initialized)
    proto_variance = np.asarray(proto_variance, dtype=np.float32)
    shadow_prototypes = np.asarray(shadow_prototypes, dtype=np.float32)
    shadow_initialized = np.asarray(shadow_initialized)
    proto_update_count = np.asarray(proto_update_count)
    count_dtype = proto_update_count.dtype

    progress = min(1.0, float(step) / max(1, WARMUP_STEPS * 10))

    key = (features.shape, float(progress))
    nc = _cache.get(key)
    if nc is None:
        nc = _build(progress)
        _cache[key] = nc

    epi = np.zeros((CPAD, W), np.float32)
    epi[0:C, 0:D] = prototypes.reshape(C, D)
    epi[0:C, D:2 * D] = shadow_prototypes.reshape(C, D)
    epi[0:C, 2 * D] = proto_variance.reshape(C)
    epi[0:C, 2 * D + 1] = proto_initialized.reshape(C).astype(np.float32)
    epi[0:C, 2 * D + 2] = shadow_initialized.reshape(C).astype(np.float32)
    epi[0:C, 2 * D + 3] = proto_update_count.reshape(C).astype(np.float32)

    in_maps = []
    for i in range(N_CORES):
        sl = slice(i * N_SHARD, (i + 1) * N_SHARD)
        lab = labels[sl].astype(np.float32).reshape(T, P).T.copy()
        in_maps.append({
            "feat": features[sl],
            "labels_t": lab,
            "epi_in": epi[i * CS:(i + 1) * CS],
        })

    res = run_bass_kernel_spmd(nc, in_maps, core_ids=list(range(N_CORES)))
    out = np.concatenate([res.results[i]["out_all"] for i in range(N_CORES)],
                         axis=0)[0:C]

    new_proto = out[:, 0:D].copy()
    new_shadow = out[:, D:2 * D].copy()
    new_var = out[:, 2 * D].copy()
    new_init = out[:, 2 * D + 1] > 0.5
    new_sinit = out[:, 2 * D + 2] > 0.5
    new_count = np.rint(out[:, 2 * D + 3]).astype(count_dtype)
    return (new_proto, new_var, new_shadow, new_init, new_sinit, new_count)


# revision 16
# speedup vs baseline: 1.0404x; 1.0404x over previous
"""Trainium2 Bass kernel for EnhancedPrototypeMemory (scatter_memory).

Strategy (8 NeuronCores, data-parallel over N):
  - Each core streams its N/8 = 16384 feature rows once from HBM with an
    on-the-fly f32->fp16 cast in the SWDGE DMA (full 128 MiB still read from
    HBM; SBUF tiles and PE operands are fp16).
  - Per 128-row tile: row sum-of-squares split across engines (a fraction of
    tiles uses ACT Square+accumulate, the rest uses a batched ACT Square into
    scratch + one batched DVE reduce) -> sqrt -> reciprocal; a one-hot matrix
    scaled by 1/||f|| is built on DVE (fused is_equal+mult tensor_scalar
    against an iota constant).
  - PE matmul (fp16 -> fp32 PSUM) accumulates onehot_scaled.T @
    [features | norm] into a [91,257] PSUM tile: cols 0..255 = normalized
    per-class sums, col 256 = per-class count (rnorm*norm = 1 per member);
    counts are re-rounded to exact integers in the epilogue.
  - The [91,257] partials are AllReduce'd across the 8 cores, then every
    core applies the identical EMA epilogue on [91,256] tiles.
  - All six epilogue state inputs are packed into ONE [91,516] input DMA and
    all six outputs into ONE [91,516] output DMA (single completion receipt).
Host only shards/packs/unpacks and converts bool/int masks to f32 and back.
"""

import numpy as np

import concourse.bacc as bacc
import concourse.mybir as mybir
import concourse.tile as tile
from concourse.bass_utils import run_bass_kernel_spmd

# Problem constants (hardcoded per contract; kernel.py must be self-contained).
N_CORES = 8
C = 91
D = 256
N_TOTAL = 131072
N_SHARD = N_TOTAL // N_CORES  # 16384
P = 128
T = N_SHARD // P  # 128 tiles per core
MOMENTUM = 0.999
WARMUP_STEPS = 200
BASE_MOM = 0.99
SHADOW_M = min(MOMENTUM + 0.0009, 0.9999)  # 0.9999

# Tunables.
GROUPS = [2, 2, 4, 8] + [16] * 7  # feature tiles per DMA group (sums to T=128)
G_MAX = max(GROUPS)
F_BUFS = 6       # feature-tile buffering

def _r_red(g):
    # tiles per group on the batched-square + DVE-reduce path; the rest use
    # ACT Square+accumulate  (~0.56 of the group keeps ACT and DVE balanced)
    return max(1, (g * 9 + 8) // 16)

# Packed epilogue state layout (both input and output): [91, 516]
#   0:256   prototypes            -> new_proto
#   256:512 shadow_prototypes     -> new_shadow
#   512     proto_variance        -> new_var
#   513     proto_initialized     -> new_init
#   514     shadow_initialized    -> new_sinit
#   515     proto_update_count    -> new_count
W = 2 * D + 4  # 516
CPAD = 96      # classes padded to 8*12 for the ReduceScatter
CS = CPAD // N_CORES  # 12-class shard per core

FP = mybir.dt.float32
FH = mybir.dt.float16
AF = mybir.ActivationFunctionType
OP = mybir.AluOpType

_cache: dict = {}


def _build(progress: float):
    nc = bacc.Bacc("TRN2", target_bir_lowering=False, debug=False,
                   num_devices=N_CORES)

    feat = nc.dram_tensor("feat", [N_SHARD, D], FP, kind="ExternalInput")
    labels_t = nc.dram_tensor("labels_t", [P, T], FP, kind="ExternalInput")
    epi_in = nc.dram_tensor("epi_in", [CS, W], FP, kind="ExternalInput")
    out_all = nc.dram_tensor("out_all", [CS, W], FP, kind="ExternalOutput")

    iota_const = nc.inline_tensor(
        np.tile(np.arange(C, dtype=np.float32), (P, 1)).astype(np.float16),
        name="iota_const")

    k_mom = float(np.float32(MOMENTUM - BASE_MOM))

    with tile.TileContext(nc) as tc:
        with (
            tc.tile_pool(name="const", bufs=1) as cpool,
            tc.tile_pool(name="feat", bufs=F_BUFS) as fpool,
            tc.tile_pool(name="oh", bufs=12) as ohpool,
            tc.tile_pool(name="stats", bufs=8) as spool,
            tc.tile_pool(name="scratch", bufs=3) as scrpool,
            tc.tile_pool(name="psum", bufs=1, space="PSUM") as pspool,
            tc.tile_pool(name="epi", bufs=1) as epool,
            tc.tile_pool(name="dram", bufs=1, space="DRAM") as dram,
        ):
            feat_r = feat[:, :].rearrange("(t p) d -> p t d", p=P)

            # First group's feature load goes first so DMA starts immediately.
            G0 = GROUPS[0]
            ftile0 = fpool.tile([P, G0, D + 1], FH, tag="ftile", name="ftile0",
                                padded_shape=[P, G_MAX, D + 1])
            nc.gpsimd.dma_start(ftile0[:, :, 0:D], feat_r[:, 0:G0, :])

            # Tiny warm-up AllGather: pays the collective entry barrier and
            # absorbs inter-core launch skew while the main body runs.
            warm_sb = cpool.tile([8, 4], FP, tag="warm_sb")
            nc.vector.memset(warm_sb[:], 1.0)
            wu_in = dram.tile([8, 4], FP, tag="wu_in")
            wu_out = dram.tile([N_CORES * 8, 4], FP, tag="wu_out")
            nc.sync.dma_start(wu_in[:], warm_sb[:])
            nc.gpsimd.collective_compute(
                "AllGather", OP.bypass,
                replica_groups=[list(range(N_CORES))],
                ins=[wu_in[:].opt()], outs=[wu_out[:].opt()])

            # One-time constants / small inputs.
            iota_sb = cpool.tile([P, C], FH, tag="iota")
            nc.sync.dma_start(iota_sb[:], iota_const[:, :])
            labels_sb = cpool.tile([P, T], FP, tag="labels")
            nc.sync.dma_start(labels_sb[:], labels_t[:, :])
            epi_sb = epool.tile([CS, W], FP, tag="epi_sb")
            nc.sync.dma_start(epi_sb[:], epi_in[:, :])

            proto_sb = epi_sb[:, 0:D]
            shadow_sb = epi_sb[:, D:2 * D]
            var_sb = epi_sb[:, 2 * D:2 * D + 1]
            init_sb = epi_sb[:, 2 * D + 1:2 * D + 2]
            sinit_sb = epi_sb[:, 2 * D + 2:2 * D + 3]
            count_sb = epi_sb[:, 2 * D + 3:2 * D + 4]

            psum = pspool.tile([C, D + 1], FP, tag="acc")

            g0 = 0
            for gi, G in enumerate(GROUPS):
                R = _r_red(G)
                if gi == 0:
                    ftile = ftile0
                else:
                    ftile = fpool.tile([P, G, D + 1], FH, tag="ftile",
                                       name=f"ftile{gi}",
                                       padded_shape=[P, G_MAX, D + 1])
                    nc.gpsimd.dma_start(ftile[:, :, 0:D],
                                        feat_r[:, g0:g0 + G, :])

                A = G - R  # tiles 0..A-1: ACT square+accumulate (per-tile
                #            release); tiles A..G-1: batched square + reduce
                # chunk 1: accumulate path
                sumsqA = spool.tile([P, max(A, 1)], FP, tag="sumsqA",
                                    padded_shape=[P, G_MAX])
                sq_scr2 = scrpool.tile([P, D], FH, tag="sq_scr2")
                for g in range(A):
                    nc.scalar.activation(
                        sq_scr2[:], ftile[:, g, 0:D], AF.Square,
                        accum_out=sumsqA[:, g:g + 1])
                # chunk 2: batched square + one reduce
                sumsqB = spool.tile([P, max(R, 1)], FP, tag="sumsqB",
                                    padded_shape=[P, G_MAX])
                sq_scr = scrpool.tile([P, R, D], FH, tag="sq_scr",
                                      padded_shape=[P, _r_red(G_MAX), D])
                nc.scalar.activation(sq_scr[:], ftile[:, A:G, 0:D], AF.Square)
                nc.vector.tensor_reduce(sumsqB[:], sq_scr[:],
                                        axis=mybir.AxisListType.X, op=OP.add)

                rnorms = []
                for (ss, lo, hi, tagix) in ((sumsqA, 0, A, "A"),
                                            (sumsqB, A, G, "B")):
                    n = hi - lo
                    nrm = spool.tile([P, n], FP, tag=f"norms{tagix}",
                                     padded_shape=[P, G_MAX])
                    nc.scalar.activation(nrm[:], ss[:, 0:n], AF.Sqrt)
                    nc.vector.tensor_copy(ftile[:, lo:hi, D:D + 1], nrm[:])
                    rn = spool.tile([P, n], FP, tag=f"rnorm{tagix}",
                                    padded_shape=[P, G_MAX])
                    nc.vector.reciprocal(rn[:], nrm[:])
                    rnorms.append((lo, rn))

                def rnorm_of(g):
                    lo, rn = rnorms[0] if g < A else rnorms[1]
                    return rn[:, g - lo:g - lo + 1]

                for g in range(G):
                    t = g0 + g
                    oh = ohpool.tile([P, C], FH, tag="oh")
                    nc.vector.tensor_scalar(
                        oh[:], iota_sb[:],
                        labels_sb[:, t:t + 1], rnorm_of(g),
                        op0=OP.is_equal, op1=OP.mult)
                    nc.tensor.matmul(
                        psum[:], oh[:], ftile[:, g, :],
                        start=(t == 0), stop=(t == T - 1))
                g0 += G

            # ---- cross-core ReduceScatter of [96, 258] padded partials ----
            # rank r receives the summed class slice [12r : 12r+12]; the
            # epilogue below runs on that 12-class shard and the host
            # concatenates the 8 output slices.
            partial = epool.tile([CPAD, D + 2], FP, tag="partial")
            nc.vector.memset(partial[:], 0.0)
            nc.scalar.copy(partial[0:C, 0:D + 1], psum[:])
            cc_in = dram.tile([CPAD, D + 2], FP, tag="cc_in")
            rs_out = dram.tile([CS, D + 2], FP, tag="rs_out")
            nc.sync.dma_start(cc_in[:], partial[:])
            nc.gpsimd.collective_compute(
                "ReduceScatter", OP.add,
                replica_groups=[list(range(N_CORES))],
                ins=[cc_in[:].opt()], outs=[rs_out[:].opt()])
            total = epool.tile([CS, D + 2], FP, tag="total")
            nc.sync.dma_start(total[:], rs_out[:, :])

            # ---- epilogue (on this core's 12-class shard) ----
            sums = total[:, 0:D]
            counts_raw = total[:, D:D + 1]

            out_sb = epool.tile([CS, W], FP, tag="out_sb")
            newp = out_sb[:, 0:D]
            news = out_sb[:, D:2 * D]
            newv = out_sb[:, 2 * D:2 * D + 1]
            newi = out_sb[:, 2 * D + 1:2 * D + 2]
            newsi = out_sb[:, 2 * D + 2:2 * D + 3]
            newc = out_sb[:, 2 * D + 3:2 * D + 4]

            def etile(tag, shape=(CS, 1)):
                return epool.tile(list(shape), FP, tag=tag, name=tag)

            # round fp16-accumulated counts to exact ints (2^23 trick)
            counts = etile("counts")
            nc.vector.tensor_scalar(counts[:], counts_raw, float(2 ** 23),
                                    float(-(2 ** 23)), op0=OP.add, op1=OP.add)

            present = etile("present")
            nc.vector.tensor_scalar(present[:], counts[:], 0.0, None,
                                    op0=OP.is_gt)
            cntc = etile("cntc")
            nc.vector.tensor_scalar(cntc[:], counts[:], 1.0, None, op0=OP.max)
            inv = etile("inv")
            nc.vector.reciprocal(inv[:], cntc[:])

            # alpha = present * (1 - init*mom); mom = BASE + k*progress*e^-var
            e = etile("e")
            nc.scalar.activation(e[:], var_sb, AF.Exp, scale=-1.0)
            momt = etile("momt")
            nc.vector.tensor_scalar(momt[:], e[:], float(k_mom * progress),
                                    init_sb, op0=OP.mult, op1=OP.mult)
            a1 = etile("a1")
            nc.vector.tensor_scalar(a1[:], init_sb, float(-BASE_MOM), 1.0,
                                    op0=OP.mult, op1=OP.add)
            a2 = etile("a2")
            nc.vector.tensor_tensor(a2[:], a1[:], momt[:], op=OP.subtract)
            alpha = etile("alpha")
            nc.vector.tensor_tensor(alpha[:], a2[:], present[:], op=OP.mult)

            d = etile("d", (CS, D))  # cls_feat - old = sums*inv - old
            nc.vector.scalar_tensor_tensor(d[:], sums, inv[:], proto_sb,
                                           op0=OP.mult, op1=OP.subtract)
            nc.vector.scalar_tensor_tensor(newp, d[:], alpha[:], proto_sb,
                                           op0=OP.mult, op1=OP.add)

            sq91 = etile("sq91", (CS, D))
            ss = etile("ss")
            nc.scalar.activation(sq91[:], d[:], AF.Square, accum_out=ss[:])
            mag = etile("mag")
            nc.scalar.activation(mag[:], ss[:], AF.Sqrt)

            w_m = etile("w_m")
            nc.vector.tensor_scalar(w_m[:], present[:], init_sb, 0.01,
                                    op0=OP.mult, op1=OP.mult)
            g2 = etile("g2")
            nc.vector.tensor_tensor(g2[:], mag[:], var_sb, op=OP.subtract)
            wg = etile("wg")
            nc.vector.tensor_tensor(wg[:], g2[:], w_m[:], op=OP.mult)
            nc.vector.tensor_tensor(newv, var_sb, wg[:], op=OP.add)

            b1 = etile("b1")
            nc.vector.tensor_scalar(b1[:], sinit_sb, float(-SHADOW_M), 1.0,
                                    op0=OP.mult, op1=OP.add)
            beta = etile("beta")
            nc.vector.tensor_tensor(beta[:], b1[:], present[:], op=OP.mult)
            d2 = etile("d2", (CS, D))
            nc.vector.tensor_tensor(d2[:], newp, shadow_sb, op=OP.subtract)
            nc.vector.scalar_tensor_tensor(news, d2[:], beta[:], shadow_sb,
                                           op0=OP.mult, op1=OP.add)

            nc.vector.tensor_tensor(newi, init_sb, present[:], op=OP.max)
            nc.vector.tensor_tensor(newsi, sinit_sb, present[:], op=OP.max)
            nc.vector.tensor_tensor(newc, count_sb, present[:], op=OP.add)

            nc.sync.dma_start(out_all[:, :], out_sb[:])

    nc.finalize()
    return nc


def kernel(features, labels, prototypes, proto_initialized, proto_variance,
           shadow_prototypes, shadow_initialized, proto_update_count, step):
    features = np.ascontiguousarray(np.asarray(features, dtype=np.float32))
    labels = np.asarray(labels)
    prototypes = np.asarray(prototypes, dtype=np.float32)
    proto_initialized = np.asarray(proto_initialized)
    proto_variance = np.asarray(proto_variance, dtype=np.float32)
    shadow_prototypes = np.asarray(shadow_prototypes, dtype=np.float32)
    shadow_initialized = np.asarray(shadow_initialized)
    proto_update_count = np.asarray(proto_update_count)
    count_dtype = proto_update_count.dtype

    progress = min(1.0, float(step) / max(1, WARMUP_STEPS * 10))

    key = (features.shape, float(progress))
    nc = _cache.get(key)
    if nc is None:
        nc = _build(progress)
        _cache[key] = nc

    epi = np.zeros((CPAD, W), np.float32)
    epi[0:C, 0:D] = prototypes.reshape(C, D)
    epi[0:C, D:2 * D] = shadow_prototypes.reshape(C, D)
    epi[0:C, 2 * D] = proto_variance.reshape(C)
    epi[0:C, 2 * D + 1] = proto_initialized.reshape(C).astype(np.float32)
    epi[0:C, 2 * D + 2] = shadow_initialized.reshape(C).astype(np.float32)
    epi[0:C, 2 * D + 3] = proto_update_count.reshape(C).astype(np.float32)

    in_maps = []
    for i in range(N_CORES):
        sl = slice(i * N_SHARD, (i + 1) * N_SHARD)
        lab = labels[sl].astype(np.float32).reshape(T, P).T.copy()
        in_maps.append({
            "feat": features[sl],
            "labels_t": lab,
            "epi_in": epi[i * CS:(i + 1) * CS],
        })

    res = run_bass_kernel_spmd(nc, in_maps, core_ids=list(range(N_CORES)))
    out = np.concatenate([res.results[i]["out_all"] for i in range(N_CORES)],
                         axis=0)[0:C]

    new_proto = out[:, 0:D].copy()
    new_shadow = out[:, D:2 * D].copy()
    new_var = out[:, 2 * D].copy()
    new_init = out[:, 2 * D + 1] > 0.5
    new_sinit = out[:, 2 * D + 2] > 0.5
    new_count = np.rint(out[:, 2 * D + 3]).astype(count_dtype)
    return (new_proto, new_var, new_shadow, new_init, new_sinit, new_count)
